# revision 8
# baseline (speedup 1.0000x reference)
"""Raw-bass (manual semaphore) equivariant-linear kernel, v2.

Math: per head h, out[b,:,h::8] = M_h^T @ x[b,:,h::8] with M_h the
512x512 3D-circulant generated from (basis@kernel)[:,h]; only 4 distinct
128x128 blocks (d = (kc-mc) mod 4). One head per NeuronCore.

v2 changes vs baseline:
  - real matmuls start as soon as w+x0 land (~9.4us) instead of after a
    fixed 10-matmul warmup chain; short N=128 dummy matmuls keep the PE
    HAM activity window busy from ~7.4us so the clock un-gates early.
  - output DRAM layout (mc, m, tb*512+n): 4KB rows. Early chunk
    (tb 0..2, 3KB rows) DMAs mid-stream; late chunk (tb 3, 1KB rows)
    DMAs per-mc as the last tile-block's psum groups retire.
  - final psum group's copy is split DVE/ACT by columns to shorten the
    last-copy -> last-DMA critical path.
  - no gpsimd block (SWDGE unused; memset moved to DVE).

Layouts (per core):
  x16 (4 tb, 128, 2048) fp16 : row p = [kc0|kc1|kc2|kc3] tokens of block tb
  w16 (128, 512) fp16        : row p = [d0|d1|d2|d3]
  o16 (4 mc, 128, 2048) fp16 : row m = [tb0|tb1|tb2|tb3] (4KB rows)
"""

import os
from contextlib import ExitStack

import numpy as np

NUM_HEADS = 8
BATCH = 32
SEQ = 512
CHAN = 512
CH = CHAN // NUM_HEADS
P = 128
NKC = 4
NMC = 4
TOK = BATCH * CH
NTB = 4
N_WARM = 10
WARM_N = 512
SPLIT = 320  # DVE takes cols [0:SPLIT] of the final copy, ACT the rest

LAST_RESULT = None
_BASS_CACHE = None


def _build_bass():
    import concourse.bass as bass
    import concourse.mybir as mybir

    fp16 = mybir.dt.float16
    fp32 = mybir.dt.float32

    nc = bass.Bass()

    x_d = nc.dram_tensor("x16", [NTB, P, NKC * 512], fp16, kind="ExternalInput")
    w_d = nc.dram_tensor("w16", [P, 4 * P], fp16, kind="ExternalInput")
    o_d = nc.dram_tensor("o16", [NMC, P, NTB * 512], fp16, kind="ExternalOutput")

    ctx = ExitStack()
    with ctx:
        XT = [
            ctx.enter_context(nc.sbuf_tensor(f"x_{tb}", [P, NKC * 512], fp16))
            for tb in range(NTB)
        ]
        warm_w = ctx.enter_context(nc.sbuf_tensor("warm_w", [P, WARM_N], fp16))
        WT = ctx.enter_context(nc.sbuf_tensor("w_all", [P, 4 * P], fp16))
        OT = [
            ctx.enter_context(nc.sbuf_tensor(f"ot_{mc}", [P, NTB * 512], fp16))
            for mc in range(NMC)
        ]
        PS = [
            ctx.enter_context(nc.psum_tensor(f"ps_{i}", [P, 512], fp32))
            for i in range(8)
        ]

        sem_mm = ctx.enter_context(nc.semaphore("mm"))
        sem_cp = ctx.enter_context(nc.semaphore("cp"))
        sem_cpa = ctx.enter_context(nc.semaphore("cpa"))
        sem_warm = ctx.enter_context(nc.semaphore("warm"))
        sem_wa = ctx.enter_context(nc.semaphore("in_wa"))
        sem_wb = ctx.enter_context(nc.semaphore("in_wb"))
        sem_x0t = ctx.enter_context(nc.semaphore("in_x0t"))
        sem_x0b = ctx.enter_context(nc.semaphore("in_x0b"))
        sem_x1 = ctx.enter_context(nc.semaphore("in_x1"))
        sem_x2 = ctx.enter_context(nc.semaphore("in_x2"))
        sem_x3 = ctx.enter_context(nc.semaphore("in_x3"))
        sem_od = ctx.enter_context(nc.semaphore("od"))

        # matmul schedule: (tb, d, mc, start, stop). d-major (weight reuse)
        # except the last tb, which is mc-major so the final psum groups
        # retire (and stream out) early.
        mm_order = []
        for tb in (0, 1, 2):
            for d in range(4):
                for mc in range(NMC):
                    mm_order.append((tb, d, mc, d == 0, d == 3))
        for mc in range(NMC):
            for d in range(4):
                mm_order.append((3, d, mc, d == 0, d == 3))

        def ps_tile(tb, mc):
            return PS[(tb % 2) * 4 + mc]

        grp_done = {}
        ngrp = 0
        for tb, d, mc, start, stop in mm_order:
            if stop:
                ngrp += 1
                grp_done[(tb, mc)] = ngrp

        # psum->sbuf copies: tb 0..2 split DVE (mc even) / ACT (mc odd);
        # tb 3 all DVE except the final group's tail columns on ACT.
        # Ordered count semaphores per engine.
        dve_copies = []  # (tb, mc, lo, hi)
        act_copies = []
        for tb in range(3):
            for mc in range(NMC):
                (dve_copies if mc % 2 == 0 else act_copies).append(
                    (tb, mc, 0, 512)
                )
        for mc in range(4):
            dve_copies.append((3, mc, 0, 512))

        cp_idx = {}
        for i, (tb, mc, lo, hi) in enumerate(dve_copies):
            cp_idx[("dve", tb, mc)] = i + 1
        for i, (tb, mc, lo, hi) in enumerate(act_copies):
            cp_idx[("act", tb, mc)] = i + 1

        with nc.Block() as block:

            @block.sync
            def _(sync):
                sync.dma_start(WT[:64], w_d[:64]).then_inc(sem_wa, 16)
                sync.dma_start(XT[0][:64], x_d[0][:64]).then_inc(sem_x0t, 16)
                sync.dma_start(XT[2][:], x_d[2]).then_inc(sem_x2, 16)
                # full-width out DMAs, mc 0,2 after their tb3 copies
                sync.wait_ge(sem_cp, cp_idx[("dve", 3, 0)])
                sync.dma_start(o_d[0], OT[0][:]).then_inc(sem_od, 16)
                sync.wait_ge(sem_cp, cp_idx[("dve", 3, 2)])
                sync.dma_start(o_d[2], OT[2][:]).then_inc(sem_od, 16)
                # final chunk mc 3
                sync.wait_ge(sem_cp, cp_idx[("dve", 3, 3)])
                sync.dma_start(o_d[3], OT[3][:]).then_inc(sem_od, 16)

            @block.scalar
            def _(scalar):
                scalar.dma_start(WT[64:], w_d[64:]).then_inc(sem_wb, 16)
                scalar.dma_start(XT[0][64:], x_d[0][64:]).then_inc(sem_x0b, 16)
                scalar.dma_start(XT[1][:], x_d[1]).then_inc(sem_x1, 16)
                # HWDGE ring holds ~256 descriptors; w+x0b+x1 fill it, so
                # gate x3's 128 descriptors on x1 completion.
                scalar.wait_ge(sem_x1, 16)
                scalar.dma_start(XT[3][:], x_d[3]).then_inc(sem_x3, 16)
                # ACT copies for tb 0..2 (mc odd)
                for tb, mc, lo, hi in act_copies:
                    scalar.wait_ge(sem_mm, grp_done[(tb, mc)])
                    nc.scalar.copy(
                        OT[mc][:, tb * 512 + lo : tb * 512 + hi],
                        ps_tile(tb, mc)[:, lo:hi],
                    ).then_inc(sem_cpa, 1)
                # full-width out DMA mc 1
                scalar.wait_ge(sem_cp, cp_idx[("dve", 3, 1)])
                scalar.dma_start(o_d[1], OT[1][:]).then_inc(sem_od, 16)


            @block.tensor
            def _(tensor):
                # HAM warm-up: short dummy matmuls so the clock-gate sees
                # PE activity from ~7.4us while inputs stream in.
                tensor.wait_ge(sem_warm, 1)
                for _ in range(N_WARM):
                    nc.tensor.matmul(
                        PS[7][:],
                        warm_w[:, :P],
                        warm_w[:],
                        start=True,
                        stop=True,
                        skip_group_check=True,
                    )
                tensor.wait_ge(sem_wa, 16)
                tensor.wait_ge(sem_wb, 16)
                tensor.wait_ge(sem_x0t, 16)
                tensor.wait_ge(sem_x0b, 16)
                xsem = {1: sem_x1, 2: sem_x2, 3: sem_x3}
                cur_tb = 0
                for tb, d, mc, start, stop in mm_order:
                    kc = (mc + d) % NKC
                    if tb != cur_tb:
                        tensor.wait_ge(xsem[tb], 16)
                        if tb >= 2:
                            # WAR: psum banks reused from tb-2; wait for
                            # that tb's copies on both engines
                            ndv = sum(
                                1
                                for (t, m, lo, hi) in dve_copies
                                if t == tb - 2
                            )
                            nac = sum(
                                1
                                for (t, m, lo, hi) in act_copies
                                if t == tb - 2
                            )
                            tensor.wait_ge(
                                sem_cp,
                                max(
                                    cp_idx[("dve", t, m)]
                                    for (t, m, lo, hi) in dve_copies
                                    if t == tb - 2
                                ),
                            )
                            if nac:
                                tensor.wait_ge(
                                    sem_cpa,
                                    max(
                                        cp_idx[("act", t, m)]
                                        for (t, m, lo, hi) in act_copies
                                        if t == tb - 2
                                    ),
                                )
                        cur_tb = tb
                    mm = nc.tensor.matmul(
                        ps_tile(tb, mc)[:],
                        WT[:, d * P : (d + 1) * P],
                        XT[tb][:, kc * 512 : (kc + 1) * 512],
                        start=start,
                        stop=stop,
                        skip_group_check=True,
                    )
                    if stop:
                        mm.then_inc(sem_mm, 1)

            @block.gpsimd
            def _(gpsimd):
                # gpsimd must have a body: BassBlock does not branch unused
                # engines to the end-of-block barrier (hangs on HW).
                gpsimd.memset(warm_w[:], 0.0).then_inc(sem_warm, 1)

            @block.vector
            def _(vector):
                for tb, mc, lo, hi in dve_copies:
                    vector.wait_ge(sem_mm, grp_done[(tb, mc)])
                    nc.vector.tensor_copy(
                        OT[mc][:, tb * 512 + lo : tb * 512 + hi],
                        ps_tile(tb, mc)[:, lo:hi],
                    ).then_inc(sem_cp, 1)

    return nc


def _weight_tiles(kexp_h):
    w3 = kexp_h.reshape(8, 8, 8)
    p = np.arange(P)
    m = np.arange(P)
    dj = ((p[:, None] // 8) % 8 - (m[None, :] // 8) % 8) % 8
    dk = (p[:, None] % 8 - m[None, :] % 8) % 8
    tiles = np.empty((4, P, P), np.float32)
    for d in range(4):
        di = (2 * d + p[:, None] // 64 - m[None, :] // 64) % 8
        tiles[d] = w3[di, dj, dk]
    return tiles


def _host_prep(x, kexp, h):
    xh = x[:, :, h::NUM_HEADS]  # (32, 512, 64)
    x_dev = (
        xh.transpose(1, 0, 2)        # (g'', b, c)
        .reshape(NKC, P, NTB, 512)   # (kc, p, tb, n)
        .transpose(2, 1, 0, 3)       # (tb, p, kc, n)
        .reshape(NTB, P, NKC * 512)
        .astype(np.float16)
    )
    w_dev = (
        _weight_tiles(kexp[:, h])    # (d, p, m)
        .transpose(1, 0, 2)          # (p, d, m)
        .reshape(P, 4 * P)
        .astype(np.float16)
    )
    return np.ascontiguousarray(x_dev), np.ascontiguousarray(w_dev)


def kernel(x, basis, kernel):
    global LAST_RESULT, _BASS_CACHE
    from concourse.bass_utils import run_bass_kernel_spmd

    x = np.ascontiguousarray(np.asarray(x, dtype=np.float32))
    kexp = np.asarray(basis, np.float32) @ np.asarray(kernel, np.float32)

    in_maps = []
    for h in range(NUM_HEADS):
        x_dev, w_dev = _host_prep(x, kexp, h)
        in_maps.append({"x16": x_dev, "w16": w_dev})

    if _BASS_CACHE is None:
        _BASS_CACHE = _build_bass()
    nc = _BASS_CACHE

    LAST_RESULT = run_bass_kernel_spmd(
        nc,
        in_maps,
        core_ids=list(range(NUM_HEADS)),
        trace=bool(int(os.environ.get("KERNEL_TRACE", "0"))),
    )

    out = np.empty((BATCH, SEQ, CHAN), np.float32)
    for h in range(NUM_HEADS):
        o_dev = LAST_RESULT.results[h]["o16"].astype(np.float32)  # (mc, m, tb*n)
        out_h = o_dev.reshape(SEQ, TOK)
        out[:, :, h::NUM_HEADS] = out_h.reshape(SEQ, BATCH, CH).transpose(1, 0, 2)
    return out
